# revision 41
# baseline (speedup 1.0000x reference)
"""Trainium2 Bass kernel for a small MLP: [N,2] -> 32 -> (8x 32) -> 1.

Strategy (data-parallel over 8 cores, batch-sharded):
  - Per core R=262144 rows, processed in 32 supertiles of 8192 rows.
  - A supertile lives in SBUF as [128 partitions, 2048 free]: 4 partition
    blocks (32 hidden channels each) x 4 free blocks (512 rows each) = 16
    groups of 512 batch rows. Group (i,f) = rows s*8192+(4i+f)*512+[0,512).
  - Each layer = 4 matmuls of [K,128]x[K,512] with BLOCK-DIAGONAL weights:
    one instruction advances 4 groups (2048 batch rows) in 512 moving rows.
  - Weights and activations are BF16 (PSUM accumulation stays fp32):
    same 1 cycle/row PE stream rate as f32r, but LDWEIGHTS gets the 2x
    fast-weight-load path and SBUF/DMA bytes halve. End-to-end rel err
    ~4e-3 (tolerance 2e-2).
  - bias+ReLU drain PSUM->SBUF: each [128,1024] psum tile (2 banks) is
    drained by a SINGLE engine (ACT activation or DVE tensor_scalar);
    tiles are assigned to engines by a least-loaded weighted balancer
    (measured per-tile cost ACT ~1115ns vs DVE ~1274ns) so both drain
    engines stay ~100% busy instead of walling on the slower one.
  - Output layer: 4 accumulating matmuls with column-shifted Wout
    placements pack all 8192 y of a supertile densely into ONE psum bank
    as [16,512] (psum accumulation over disjoint output partitions), so
    the final drain is FD=512 instead of 2048; bout added on host.
"""

import numpy as np

N = 2097152
H = 32
L = 8
N_CORES = 8
R = N // N_CORES          # 262144 rows per core
FB = 512                  # rows per group
ST_ROWS = 16 * FB         # 8192 rows per supertile
N_ST = R // ST_ROWS       # 32 supertiles per core

# Measured per-tile drain costs (ns) used for balancing, by FD elems.
def _act_ns(fd):
    return (172 + fd) / 1.2 * 1.12


def _dve_ns(fd):
    return (120 + fd) / 0.96 * 1.07

_CACHE = {}


def _build_nc(n_st=N_ST):
    import concourse.tile as tile
    from concourse import bacc, mybir

    f32 = mybir.dt.float32
    bf16 = mybir.dt.bfloat16

    nc = bacc.Bacc(None, target_bir_lowering=False)
    xt_d = nc.dram_tensor("xt", [8, n_st, 2048], bf16, kind="ExternalInput")
    wm_d = nc.dram_tensor("wmat", [128, 2176], bf16, kind="ExternalInput")
    wb_d = nc.dram_tensor("wbias", [128, 9], f32, kind="ExternalInput")
    out_d = nc.dram_tensor("out", [n_st, 16, 512], f32,
                           kind="ExternalOutput")

    relu = mybir.ActivationFunctionType.Relu
    alu_add = mybir.AluOpType.add
    alu_max = mybir.AluOpType.max

    # Weighted least-loaded assignment of drain tiles to ACT / DVE.
    load = {"act": 0.0, "dve": 0.0}

    def pick_engine(fd):
        e = "act" if load["act"] + _act_ns(fd) <= load["dve"] + _dve_ns(fd) \
            else "dve"
        load[e] += _act_ns(fd) if e == "act" else _dve_ns(fd)
        return e

    with tile.TileContext(nc) as tc:
        with tc.tile_pool(name="wpool", bufs=1) as wpool, \
             tc.tile_pool(name="xpool", bufs=4) as xpool, \
             tc.tile_pool(name="hpool", bufs=4) as hpool, \
             tc.tile_pool(name="pspool", bufs=2, space="PSUM") as pspool:
            # Load layer-0's weight columns + biases first so compute can
            # start while the bulk of the weights stream in behind them
            # (subtile deps: layer-l matmuls wait only on their columns).
            w = wpool.tile([128, 2176], bf16)
            nc.sync.dma_start(out=w[:, 0:128], in_=wm_d[:, 0:128])
            wb = wpool.tile([128, 9], f32)
            nc.sync.dma_start(out=wb[:], in_=wb_d[:, :])

            def drain_relu(dst, src, bias, fd):
                if pick_engine(fd) == "act":
                    nc.scalar.activation(dst, src, relu, bias=bias)
                else:
                    nc.vector.tensor_scalar(dst, src, bias, 0.0,
                                            alu_add, alu_max)

            def drain_copy(dst, src, fd):
                if pick_engine(fd) == "act":
                    nc.scalar.copy(dst, src)
                else:
                    nc.vector.tensor_scalar_add(dst, src, 0.0)

            # Output layer: 4 accumulating matmuls with column-shifted
            # Wout placements land f-block j's y on partitions {4i+j} of
            # ONE psum bank -> a single FD=512 drain + one [16,512] DMA
            # per supertile (vs a redundant [128,2048] drain).
            def emit_l9(g, stash):
                psY = pspool.tile([128, 512], f32, tag="ps", bufs=4)
                for q, (s, h01, h23) in enumerate(stash):
                    for j in range(4):
                        c9 = 1152 + 64 * (4 * q + j)
                        w9 = w[0:128, c9:c9 + 16]
                        hsrc = h01 if j < 2 else h23
                        c0 = 512 * (j % 2)
                        nc.tensor.matmul(psY[0:16, :], w9,
                                         hsrc[0:128, c0:c0 + 512],
                                         start=(q == 0 and j == 0),
                                         stop=(q == 0 and j == 3))
                ho = hpool.tile([128, 512], f32)
                drain_copy(ho[0:16, :], psY[0:16, :], 512)
                nc.sync.dma_start(out=out_d[g, :, :], in_=ho[0:16, :])

            # Each group-layer uses TWO psum tiles (2 banks each) and TWO
            # SBUF out tiles, each drained by one engine via the balancer.
            def layer(s, l, h01, h23, kdim):
                psL = pspool.tile([128, 1024], f32, tag="ps", bufs=4)
                psR = pspool.tile([128, 1024], f32, tag="ps", bufs=4)
                wcol = w[0:kdim, 128 * l:128 * (l + 1)]
                nc.tensor.matmul(psL[:, 0:512], wcol, h01[0:kdim, 0:512])
                nc.tensor.matmul(psL[:, 512:1024], wcol,
                                 h01[0:kdim, 512:1024])
                bias = wb[:, l:l + 1]
                hn01 = hpool.tile([128, 1024], bf16)
                drain_relu(hn01[:], psL[:], bias, 1024)
                nc.tensor.matmul(psR[:, 0:512], wcol, h23[0:kdim, 0:512])
                nc.tensor.matmul(psR[:, 512:1024], wcol,
                                 h23[0:kdim, 512:1024])
                hn23 = hpool.tile([128, 1024], bf16)
                drain_relu(hn23[:], psR[:], bias, 1024)
                return hn01, hn23

            # Sliding-window pipeline, 4 supertiles in flight at staggered
            # layers: while supertile A's layer-l drain runs on ACT/DVE,
            # the PE does the other supertiles' matmuls. A new supertile
            # enters as soon as one retires, so there is no group barrier.
            # x loads are prefetched LOOKAHEAD supertiles ahead of entry so
            # the ramp isn't gated by the one-at-a-time DMA ring drip.
            LOOKAHEAD = 6
            xs_loaded = {}

            def load_x(s):
                if s in xs_loaded or not (0 <= s < n_st):
                    return
                x01 = xpool.tile([8, 1024], bf16, tag="x01", bufs=LOOKAHEAD)
                nc.sync.dma_start(out=x01[:], in_=xt_d[:, s, 0:1024])
                x23 = xpool.tile([8, 1024], bf16, tag="x23", bufs=LOOKAHEAD)
                nc.sync.dma_start(out=x23[:], in_=xt_d[:, s, 1024:2048])
                xs_loaded[s] = (x01, x23)

            def enter(s):
                load_x(s)
                load_x(s + LOOKAHEAD - 1)
                x01, x23 = xs_loaded.pop(s)
                return [s, 0, x01, x23]

            DEPTH = 4
            load_x(0)
            load_x(1)
            flight = [enter(0)]
            nc.sync.dma_start(out=w[:, 128:2176], in_=wm_d[:, 128:2176])
            for s in range(2, LOOKAHEAD):
                load_x(s)
            stash = []
            nxt = 1
            while flight or nxt < n_st:
                if nxt < n_st and len(flight) < DEPTH:
                    flight.append(enter(nxt))
                    nxt += 1
                for f in list(flight):
                    s, l, h01, h23 = f
                    if l == 9:
                        flight.remove(f)
                        stash.append((s, h01, h23))
                        if len(stash) == 1:
                            emit_l9(stash[0][0], stash)
                            stash = []
                        continue
                    hn01, hn23 = layer(s, l, h01, h23, 8 if l == 0 else 128)
                    f[1], f[2], f[3] = l + 1, hn01, hn23
    nc.finalize()
    return nc


def _to_bf16(a):
    import ml_dtypes
    return np.asarray(a, np.float32).astype(ml_dtypes.bfloat16)


def _prep_core_inputs(x_shard, wmat, wbias):
    # xt[2i+c, s, 512f+r] = x_shard[s*8192 + (4i+f)*512 + r, c]
    n_st = x_shard.shape[0] // ST_ROWS
    xs = np.ascontiguousarray(x_shard).reshape(n_st, 4, 4, FB, 2)
    xt = np.ascontiguousarray(xs.transpose(1, 4, 0, 2, 3)).reshape(
        8, n_st, 2048)
    return {"xt": xt, "wmat": wmat, "wbias": wbias}


def _pack_weights(W0, b0, Wh, bh, Wout):
    # Block-diagonal lhsT per layer, 128 cols each:
    #   l=0:    wmat[2i+c, 32i+m]       = W0[m, c]        (K=8 rows used)
    #   l=1..8: wmat[32i+k, 128l+32i+m] = Wh[l-1][m, k]
    #   l=9:    wmat[32i+k, 1152+64(4q+j)+16q+4i+j] = Wout[0, k] (lhsT for
    #           supertile-slot q, f-block j: y lands on partition 16q+4i+j)
    wmat = np.zeros((128, 1152 + 1024), dtype=np.float32)
    wbias = np.zeros((128, 9), dtype=np.float32)
    for i in range(4):
        wmat[2 * i:2 * i + 2, 32 * i:32 * i + 32] = W0.T
        for hl in range(L):
            wmat[32 * i:32 * i + 32,
                 128 * (hl + 1) + 32 * i:128 * (hl + 1) + 32 * i + 32] = \
                Wh[hl].T
        for q in range(4):
            for j in range(4):
                wmat[32 * i:32 * i + 32,
                     1152 + 64 * (4 * q + j) + 16 * q + 4 * i + j] = Wout[0, :]
        wbias[32 * i:32 * i + 32, 0] = b0
        for hl in range(L):
            wbias[32 * i:32 * i + 32, 1 + hl] = bh[hl]
    return _to_bf16(wmat), wbias


def kernel(x, W0, b0, Wh, bh, Wout, bout):
    from concourse import bass_utils

    if "nc" not in _CACHE:
        _CACHE["nc"] = _build_nc()
    nc = _CACHE["nc"]

    wmat, wbias = _pack_weights(np.asarray(W0, np.float32),
                                np.asarray(b0, np.float32),
                                np.asarray(Wh, np.float32),
                                np.asarray(bh, np.float32),
                                np.asarray(Wout, np.float32))
    x = _to_bf16(x)
    in_maps = [_prep_core_inputs(x[c * R:(c + 1) * R], wmat, wbias)
               for c in range(N_CORES)]

    res = bass_utils.run_bass_kernel_spmd(nc, in_maps, list(range(N_CORES)))
    out = np.concatenate([r["out"].reshape(R) for r in res.results])
    return (out.reshape(N, 1) + np.float32(bout[0])).astype(np.float32)


# revision 42
# speedup vs baseline: 1.0056x; 1.0056x over previous
"""Trainium2 Bass kernel for a small MLP: [N,2] -> 32 -> (8x 32) -> 1.

Strategy (data-parallel over 8 cores, batch-sharded):
  - Per core R=262144 rows, processed in 32 supertiles of 8192 rows.
  - A supertile lives in SBUF as [128 partitions, 2048 free]: 4 partition
    blocks (32 hidden channels each) x 4 free blocks (512 rows each) = 16
    groups of 512 batch rows. Group (i,f) = rows s*8192+(4i+f)*512+[0,512).
  - Each layer = 4 matmuls of [K,128]x[K,512] with BLOCK-DIAGONAL weights:
    one instruction advances 4 groups (2048 batch rows) in 512 moving rows.
  - Weights and activations are BF16 (PSUM accumulation stays fp32):
    same 1 cycle/row PE stream rate as f32r, but LDWEIGHTS gets the 2x
    fast-weight-load path and SBUF/DMA bytes halve. End-to-end rel err
    ~4e-3 (tolerance 2e-2).
  - bias+ReLU drain PSUM->SBUF: each [128,1024] psum tile (2 banks) is
    drained by a SINGLE engine (ACT activation or DVE tensor_scalar);
    tiles are assigned to engines by a least-loaded weighted balancer
    (measured per-tile cost ACT ~1115ns vs DVE ~1274ns) so both drain
    engines stay ~100% busy instead of walling on the slower one.
  - Output layer: 4 accumulating matmuls with column-shifted Wout
    placements pack all 8192 y of a supertile densely into ONE psum bank
    as [16,512] (psum accumulation over disjoint output partitions), so
    the final drain is FD=512 instead of 2048; bout added on host.
"""

import numpy as np

N = 2097152
H = 32
L = 8
N_CORES = 8
R = N // N_CORES          # 262144 rows per core
FB = 512                  # rows per group
ST_ROWS = 16 * FB         # 8192 rows per supertile
N_ST = R // ST_ROWS       # 32 supertiles per core

# Measured per-tile drain costs (ns) used for balancing, by FD elems.
def _act_ns(fd):
    return (172 + fd) / 1.2 * 1.12


def _dve_ns(fd):
    return (120 + fd) / 0.96 * 1.07

_CACHE = {}


def _build_nc(n_st=N_ST):
    import concourse.tile as tile
    from concourse import bacc, mybir

    f32 = mybir.dt.float32
    bf16 = mybir.dt.bfloat16

    nc = bacc.Bacc(None, target_bir_lowering=False)
    xt_d = nc.dram_tensor("xt", [8, n_st, 2048], bf16, kind="ExternalInput")
    wm_d = nc.dram_tensor("wmat", [128, 2176], bf16, kind="ExternalInput")
    wb_d = nc.dram_tensor("wbias", [128, 9], f32, kind="ExternalInput")
    out_d = nc.dram_tensor("out", [n_st, 16, 512], f32,
                           kind="ExternalOutput")

    relu = mybir.ActivationFunctionType.Relu
    alu_add = mybir.AluOpType.add
    alu_max = mybir.AluOpType.max

    # Weighted least-loaded assignment of drain tiles to ACT / DVE.
    load = {"act": 0.0, "dve": 0.0}

    def pick_engine(fd):
        e = "act" if load["act"] + _act_ns(fd) <= load["dve"] + _dve_ns(fd) \
            else "dve"
        load[e] += _act_ns(fd) if e == "act" else _dve_ns(fd)
        return e

    with tile.TileContext(nc) as tc:
        with tc.tile_pool(name="wpool", bufs=1) as wpool, \
             tc.tile_pool(name="xpool", bufs=4) as xpool, \
             tc.tile_pool(name="hpool", bufs=4) as hpool, \
             tc.tile_pool(name="pspool", bufs=2, space="PSUM") as pspool:
            # Load layer-0's weight columns + biases first so compute can
            # start while the bulk of the weights stream in behind them
            # (subtile deps: layer-l matmuls wait only on their columns).
            w = wpool.tile([128, 2176], bf16)
            nc.sync.dma_start(out=w[:, 0:128], in_=wm_d[:, 0:128])
            wb = wpool.tile([128, 9], f32)
            nc.sync.dma_start(out=wb[:], in_=wb_d[:, :])

            def drain_relu(dst, src, bias, fd):
                if pick_engine(fd) == "act":
                    nc.scalar.activation(dst, src, relu, bias=bias)
                else:
                    nc.vector.tensor_scalar(dst, src, bias, 0.0,
                                            alu_add, alu_max)

            def drain_copy(dst, src, fd):
                if pick_engine(fd) == "act":
                    nc.scalar.copy(dst, src)
                else:
                    nc.vector.tensor_scalar_add(dst, src, 0.0)

            # Output layer: 4 accumulating matmuls with column-shifted
            # Wout placements land f-block j's y on partitions {4i+j} of
            # ONE psum bank -> a single FD=512 drain + one [16,512] DMA
            # per supertile (vs a redundant [128,2048] drain).
            def emit_l9(g, stash):
                psY = pspool.tile([128, 512], f32, tag="ps", bufs=4)
                for q, (s, h01, h23) in enumerate(stash):
                    for j in range(4):
                        c9 = 1152 + 64 * (4 * q + j)
                        w9 = w[0:128, c9:c9 + 16]
                        hsrc = h01 if j < 2 else h23
                        c0 = 512 * (j % 2)
                        nc.tensor.matmul(psY[0:16, :], w9,
                                         hsrc[0:128, c0:c0 + 512],
                                         start=(q == 0 and j == 0),
                                         stop=(q == 0 and j == 3))
                ho = hpool.tile([128, 512], f32)
                drain_copy(ho[0:16, :], psY[0:16, :], 512)
                nc.sync.dma_start(out=out_d[g, :, :], in_=ho[0:16, :])

            # Each group-layer uses TWO psum tiles (2 banks each) and TWO
            # SBUF out tiles, each drained by one engine via the balancer.
            def layer(s, l, h01, h23, kdim):
                psL = pspool.tile([128, 1024], f32, tag="ps", bufs=4)
                psR = pspool.tile([128, 1024], f32, tag="ps", bufs=4)
                wcol = w[0:kdim, 128 * l:128 * (l + 1)]
                nc.tensor.matmul(psL[:, 0:512], wcol, h01[0:kdim, 0:512])
                nc.tensor.matmul(psL[:, 512:1024], wcol,
                                 h01[0:kdim, 512:1024])
                bias = wb[:, l:l + 1]
                hn01 = hpool.tile([128, 1024], bf16)
                drain_relu(hn01[:], psL[:], bias, 1024)
                nc.tensor.matmul(psR[:, 0:512], wcol, h23[0:kdim, 0:512])
                nc.tensor.matmul(psR[:, 512:1024], wcol,
                                 h23[0:kdim, 512:1024])
                hn23 = hpool.tile([128, 1024], bf16)
                drain_relu(hn23[:], psR[:], bias, 1024)
                return hn01, hn23

            # Sliding-window pipeline, 4 supertiles in flight at staggered
            # layers: while supertile A's layer-l drain runs on ACT/DVE,
            # the PE does the other supertiles' matmuls. A new supertile
            # enters as soon as one retires, so there is no group barrier.
            # x loads are prefetched LOOKAHEAD supertiles ahead of entry so
            # the ramp isn't gated by the one-at-a-time DMA ring drip.
            LOOKAHEAD = 6
            xs_loaded = {}

            def load_x(s):
                if s in xs_loaded or not (0 <= s < n_st):
                    return
                # Ramp burst rides the otherwise-idle scalar HWDGE ring.
                q = nc.scalar if s < LOOKAHEAD else nc.sync
                x01 = xpool.tile([8, 1024], bf16, tag="x01", bufs=LOOKAHEAD)
                q.dma_start(out=x01[:], in_=xt_d[:, s, 0:1024])
                x23 = xpool.tile([8, 1024], bf16, tag="x23", bufs=LOOKAHEAD)
                q.dma_start(out=x23[:], in_=xt_d[:, s, 1024:2048])
                xs_loaded[s] = (x01, x23)

            def enter(s):
                load_x(s)
                load_x(s + LOOKAHEAD - 1)
                x01, x23 = xs_loaded.pop(s)
                return [s, 0, x01, x23]

            DEPTH = 4
            load_x(0)
            load_x(1)
            flight = [enter(0)]
            nc.sync.dma_start(out=w[:, 128:2176], in_=wm_d[:, 128:2176])
            for s in range(2, LOOKAHEAD):
                load_x(s)
            stash = []
            nxt = 1
            while flight or nxt < n_st:
                if nxt < n_st and len(flight) < DEPTH:
                    flight.append(enter(nxt))
                    nxt += 1
                for f in list(flight):
                    s, l, h01, h23 = f
                    if l == 9:
                        flight.remove(f)
                        stash.append((s, h01, h23))
                        if len(stash) == 1:
                            emit_l9(stash[0][0], stash)
                            stash = []
                        continue
                    hn01, hn23 = layer(s, l, h01, h23, 8 if l == 0 else 128)
                    f[1], f[2], f[3] = l + 1, hn01, hn23
    nc.finalize()
    return nc


def _to_bf16(a):
    import ml_dtypes
    return np.asarray(a, np.float32).astype(ml_dtypes.bfloat16)


def _prep_core_inputs(x_shard, wmat, wbias):
    # xt[2i+c, s, 512f+r] = x_shard[s*8192 + (4i+f)*512 + r, c]
    n_st = x_shard.shape[0] // ST_ROWS
    xs = np.ascontiguousarray(x_shard).reshape(n_st, 4, 4, FB, 2)
    xt = np.ascontiguousarray(xs.transpose(1, 4, 0, 2, 3)).reshape(
        8, n_st, 2048)
    return {"xt": xt, "wmat": wmat, "wbias": wbias}


def _pack_weights(W0, b0, Wh, bh, Wout):
    # Block-diagonal lhsT per layer, 128 cols each:
    #   l=0:    wmat[2i+c, 32i+m]       = W0[m, c]        (K=8 rows used)
    #   l=1..8: wmat[32i+k, 128l+32i+m] = Wh[l-1][m, k]
    #   l=9:    wmat[32i+k, 1152+64(4q+j)+16q+4i+j] = Wout[0, k] (lhsT for
    #           supertile-slot q, f-block j: y lands on partition 16q+4i+j)
    wmat = np.zeros((128, 1152 + 1024), dtype=np.float32)
    wbias = np.zeros((128, 9), dtype=np.float32)
    for i in range(4):
        wmat[2 * i:2 * i + 2, 32 * i:32 * i + 32] = W0.T
        for hl in range(L):
            wmat[32 * i:32 * i + 32,
                 128 * (hl + 1) + 32 * i:128 * (hl + 1) + 32 * i + 32] = \
                Wh[hl].T
        for q in range(4):
            for j in range(4):
                wmat[32 * i:32 * i + 32,
                     1152 + 64 * (4 * q + j) + 16 * q + 4 * i + j] = Wout[0, :]
        wbias[32 * i:32 * i + 32, 0] = b0
        for hl in range(L):
            wbias[32 * i:32 * i + 32, 1 + hl] = bh[hl]
    return _to_bf16(wmat), wbias


def kernel(x, W0, b0, Wh, bh, Wout, bout):
    from concourse import bass_utils

    if "nc" not in _CACHE:
        _CACHE["nc"] = _build_nc()
    nc = _CACHE["nc"]

    wmat, wbias = _pack_weights(np.asarray(W0, np.float32),
                                np.asarray(b0, np.float32),
                                np.asarray(Wh, np.float32),
                                np.asarray(bh, np.float32),
                                np.asarray(Wout, np.float32))
    x = _to_bf16(x)
    in_maps = [_prep_core_inputs(x[c * R:(c + 1) * R], wmat, wbias)
               for c in range(N_CORES)]

    res = bass_utils.run_bass_kernel_spmd(nc, in_maps, list(range(N_CORES)))
    out = np.concatenate([r["out"].reshape(R) for r in res.results])
    return (out.reshape(N, 1) + np.float32(bout[0])).astype(np.float32)
